# revision 18
# baseline (speedup 1.0000x reference)
"""Trainium2 Bass kernel for the ragged-sequence attention problem.

Math (per batch row):
    u      = tanh(h @ W.T + b)          h: [L, H]
    s      = u @ u_s                    masked to length, then softmax
    v      = sum_l alpha_l * h_l

Strategy (v2 — ragged, transpose-free):
  - Length-aware schedule: rows are LPT-packed onto 8 cores (32 rows each,
    balanced by tile count T_b = ceil(len/128)), each core's rows sorted by
    T descending.  The SPMD program processes slot i with
    T_seq[i] = max over cores of that rank's T, so one program serves all
    cores; padding overhead is ~2% of tiles.  ~293 tiles/core vs 512 dense.
  - h is pre-cast to bf16 and padded to 256 channels on the host; the
    DMA XBAR-transpose loads it c-major ([c, l] tiles, two 128-channel
    chunks) straight from DRAM — no PE transposes, no PSUM evacuation,
    no on-device f32->bf16 cast.
  - u-matmul: 4 matmuls per 512-l group (2 k-chunks x 2 m-chunks),
    stationary = W.T blocks, moving = ht chunks.  tanh+bias on ScalarE
    into a [128, 1024] bf16 tile (both m-chunks).
  - scores: stationary u_s column, moving ut; groups land on psum
    partitions 0/32/64/96 via tile_position.  The length mask is folded in
    as a K=1 matmul adding a -1e30 log-mask row, skipped for groups that
    are provably full on every core.  One Exp ACT over ps[0:97] per row
    yields unnormalized alpha rows directly from PSUM.
  - v: alpha row broadcast across partitions (GpSimd) then a DVE
    tensor_tensor_reduce against each ht chunk accumulates v per group.
    Host sums the shipped alpha rows for the softmax denominator, divides,
    and unpermutes.
  - PE stream is software-pipelined: scores lag the u-matmuls by one
    group, tails (exp/v) lag by one row, so the in-order queues never
    stall on cross-engine round trips.
"""

import math
import sys

import numpy as np

sys.path.insert(0, "/opt/trn_rl_repo")

import ml_dtypes  # noqa: E402

import concourse.bass as bass  # noqa: E402
import concourse.mybir as mybir  # noqa: E402
import concourse.tile as tile  # noqa: E402
from concourse.bass_utils import run_bass_kernel_spmd  # noqa: E402
import bass_rust as _br  # noqa: E402

N_CORES = 8
B, L, H = 256, 2048, 240
BPC = B // N_CORES        # batch rows per core
HP = 256                  # h channels padded (two 128 chunks)
H1 = H - 128              # 112 real channels in chunk 1
NG = 4                    # max l-groups of 512
GSZ = 512
F32 = mybir.dt.float32
BF16 = mybir.dt.bfloat16
AF = mybir.ActivationFunctionType
ALU = mybir.AluOpType
AX = mybir.AxisListType
BF16NP = ml_dtypes.bfloat16

_MAXW = 1  # sync waits kept on an instruction; the rest move to nops


class _TC(tile.TileContext):
    """Walrus in this container caps sync-wait commands per instruction
    ("Too many sync wait commands"), but Tile freely attaches one wait per
    producer semaphore.  After scheduling, hoist excess waits onto dedicated
    single-wait nops inserted just before the instruction on its engine."""

    def schedule_and_allocate(self, validate_deps=False):
        ret = super().schedule_and_allocate(validate_deps)
        self._split_excess_waits()
        return ret

    def _split_excess_waits(self):
        nc = self.nc
        n_split = 0
        for fn in nc.m.functions:
            for bb in fn.blocks:
                insts = bb.instructions
                i = 0
                while i < len(insts):
                    inst = insts[i]
                    si = getattr(inst, "sync_info", None)
                    waits = list(si.on_wait) if si is not None else []
                    if len(waits) > _MAXW:
                        si.on_wait = waits[-_MAXW:]
                        inst.sync_info = si
                        for w in waits[:-_MAXW]:
                            nop = mybir.InstNoOp(
                                name=f"waitsplit-{n_split}", ins=[], outs=[])
                            n_split += 1
                            nop.engine = inst.engine
                            nop.sync_info = _br.SyncInfo(
                                on_wait=[w], on_update=[])
                            nc.register_instruction(nop, overwrite=True)
                            insts.insert(i, nop)
                            i += 1
                    i += 1


def _schedule(lens):
    """LPT-pack rows onto cores; per-core descending by tile count.
    Returns (perm [8][32] row ids, T_seq [32], full [32][4] bools)."""
    lens = np.asarray(lens).astype(np.int64)
    T = np.ceil(lens / 128).astype(np.int64)
    order = np.argsort(-T, kind="stable")
    perm = [[] for _ in range(N_CORES)]
    loads = [0] * N_CORES
    for idx in order:
        cand = sorted(range(N_CORES), key=lambda c: (loads[c], len(perm[c])))
        for c in cand:
            if len(perm[c]) < BPC:
                perm[c].append(int(idx))
                loads[c] += int(T[idx])
                break
    for c in range(N_CORES):
        perm[c].sort(key=lambda r: -int(T[r]))
    T_seq = [max(int(T[perm[c][i]]) for c in range(N_CORES))
             for i in range(BPC)]
    minlen = [min(int(lens[perm[c][i]]) for c in range(N_CORES))
              for i in range(BPC)]
    full = [[GSZ * (g + 1) <= minlen[i] for g in range(NG)]
            for i in range(BPC)]
    return perm, tuple(T_seq), tuple(tuple(f) for f in full)


def build(T_seq, full):
    nc = bass.Bass("TRN2", target_bir_lowering=False, debug=False,
                   num_devices=N_CORES)
    h_d = nc.declare_dram_parameter("h", [BPC, L, HP], BF16, isOutput=False)
    w0_d = nc.declare_dram_parameter("wtb0", [128, HP], BF16, isOutput=False)
    w1_d = nc.declare_dram_parameter("wtb1", [128, HP], BF16, isOutput=False)
    u0_d = nc.declare_dram_parameter("usb0", [128, BPC], BF16, isOutput=False)
    u1_d = nc.declare_dram_parameter("usb1", [128, BPC], BF16, isOutput=False)
    b0_d = nc.declare_dram_parameter("b0", [128, 1], F32, isOutput=False)
    b1_d = nc.declare_dram_parameter("b1", [128, 1], F32, isOutput=False)
    m_d = nc.declare_dram_parameter("logm", [BPC, NG, GSZ], BF16,
                                    isOutput=False)
    ov_d = nc.declare_dram_parameter("ov", [BPC, 128, 2], F32, isOutput=True)
    oa_d = nc.declare_dram_parameter("oa", [BPC, NG, GSZ], BF16, isOutput=True)

    with _TC(nc) as tc:
        with (
            tc.tile_pool(name="consts", bufs=1) as cp,
            tc.tile_pool(name="ht", bufs=3) as htp,
            tc.tile_pool(name="ut", bufs=3) as utp,
            tc.tile_pool(name="al", bufs=2) as alp,
            tc.tile_pool(name="sc", bufs=2) as scp,
            tc.tile_pool(name="sm", bufs=3) as smp,
            tc.tile_pool(name="pu", bufs=2, space="PSUM") as pup,
            tc.tile_pool(name="ps", bufs=2, space="PSUM") as psp,
            tc.tile_pool(name="pb", bufs=2, space="PSUM") as pbp,
        ):
            wtb0 = cp.tile([128, HP], BF16)
            wtb1 = cp.tile([128, HP], BF16)
            usb0 = cp.tile([128, BPC], BF16)
            usb1 = cp.tile([128, BPC], BF16)
            b0 = cp.tile([128, 1], F32)
            b1 = cp.tile([128, 1], F32)
            one1 = cp.tile([1, 1], BF16)
            onesT = cp.tile([97, 128], BF16)
            nc.sync.dma_start(wtb0[:], w0_d.ap()[:, :])
            nc.sync.dma_start(wtb1[:], w1_d.ap()[:, :])
            nc.sync.dma_start(usb0[:], u0_d.ap()[:, :])
            nc.sync.dma_start(usb1[:], u1_d.ap()[:, :])
            nc.sync.dma_start(b0[:], b0_d.ap()[:, :])
            nc.sync.dma_start(b1[:], b1_d.ap()[:, :])
            nc.gpsimd.memset(one1[:], 1.0)
            nc.gpsimd.memset(onesT[:], 1.0)

            # ---- per-row emission helpers -------------------------------
            def emit_row_loads(i, T):
                ht = htp.tile([128, 2 * L], BF16, tag="ht")
                nc.sync.dma_start(ht[:, 0:128 * T],
                                  h_d.ap()[i, 0:128 * T, 0:128],
                                  transpose=True)
                nc.sync.dma_start(ht[:, L:L + 128 * T],
                                  h_d.ap()[i, 0:128 * T, 128:HP],
                                  transpose=True)
                logm = smp.tile([1, NG * GSZ], BF16, tag="logm")
                nc.gpsimd.dma_start(
                    logm[:],
                    m_d.ap()[i:i + 1].rearrange("o g c -> o (g c)"))
                return ht, logm

            def emit_u(st, g):
                (i, T, ht, logm, ps, uts, pus) = st
                N = min(GSZ, 128 * T - g * GSZ)
                gs = slice(g * GSZ, g * GSZ + N)
                gs1 = slice(L + g * GSZ, L + g * GSZ + N)
                pu0 = pup.tile([128, GSZ], F32, tag="pu0")
                pu1 = pup.tile([128, GSZ], F32, tag="pu1")
                nc.tensor.matmul(pu0[:, 0:N], wtb0[:, 0:128], ht[:, gs],
                                 start=True, stop=False)
                nc.tensor.matmul(pu0[:, 0:N], wtb1[:, 0:128], ht[:, gs1],
                                 start=False, stop=True)
                nc.tensor.matmul(pu1[:, 0:N], wtb0[:, 128:HP], ht[:, gs],
                                 start=True, stop=False)
                nc.tensor.matmul(pu1[:, 0:N], wtb1[:, 128:HP], ht[:, gs1],
                                 start=False, stop=True)
                pus[g] = (pu0, pu1, N)

            def emit_tanh(st, g):
                (i, T, ht, logm, ps, uts, pus) = st
                pu0, pu1, N = pus[g]
                ut = utp.tile([128, 2 * GSZ], BF16, tag="ut")
                nc.scalar.activation(ut[:, 0:N], pu0[:, 0:N], AF.Tanh,
                                     bias=b0[:])
                nc.scalar.activation(ut[:, GSZ:GSZ + N], pu1[:, 0:N], AF.Tanh,
                                     bias=b1[:])
                uts[g] = (ut, N)

            def emit_scores(st, g):
                (i, T, ht, logm, ps, uts, pus) = st
                ut, N = uts[g]
                tp = (0, 32 * g)
                isfull = full[i][g]
                nc.tensor.matmul(ps[32 * g:32 * g + 1, 0:N],
                                 usb0[:, i:i + 1], ut[:, 0:N],
                                 start=True, stop=False, tile_position=tp)
                nc.tensor.matmul(ps[32 * g:32 * g + 1, 0:N],
                                 usb1[:, i:i + 1], ut[:, GSZ:GSZ + N],
                                 start=False, stop=isfull, tile_position=tp)
                if not isfull:
                    nc.tensor.matmul(ps[32 * g:32 * g + 1, 0:N],
                                     one1[:],
                                     logm[:, g * GSZ:g * GSZ + N],
                                     start=False, stop=True, tile_position=tp)

            def emit_tail(st):
                (i, T, ht, logm, ps, uts, pus) = st
                G = (T + 3) // 4
                al = alp.tile([97, GSZ], BF16, tag="al")
                nc.scalar.activation(al[:], ps[0:97, :], AF.Exp)
                vg = scp.tile([128, 2 * NG], F32, tag="vg")
                for g in range(G):
                    N = min(GSZ, 128 * T - g * GSZ)
                    ab = pbp.tile([128, GSZ], F32, tag="ab")
                    nc.tensor.matmul(ab[:, 0:N],
                                     onesT[32 * g:32 * g + 1, :],
                                     al[32 * g:32 * g + 1, 0:N],
                                     start=True, stop=True,
                                     tile_position=(32 * g, 0))
                    # both h chunks in one pass: free dims [chunk=2, l=N]
                    hview = ht[:].rearrange("p (k l) -> p k l", k=2)
                    prod = smp.tile([128, 2 * GSZ], BF16, tag="prod")
                    pview = prod[:].rearrange("p (k l) -> p k l", k=2)
                    nc.vector.tensor_mul(
                        pview[:, :, 0:N],
                        hview[:, :, g * GSZ:g * GSZ + N],
                        ab[:, 0:N].rearrange(
                            "p (o l) -> p o l", o=1).to_broadcast((128, 2, N)))
                    nc.vector.tensor_reduce(
                        vg[:].rearrange("p (g k) -> p g k", k=2)[:, g, :],
                        pview[:, :, 0:N], AX.X, ALU.add)
                    nc.sync.dma_start(oa_d.ap()[i, g:g + 1, 0:N],
                                      al[32 * g:32 * g + 1, 0:N])
                vfin = scp.tile([128, 2], F32, tag="vfin")
                nc.vector.tensor_reduce(
                    vfin[:],
                    vg[:].rearrange("p (g k) -> p k g", k=2)[:, :, 0:G],
                    AX.X, ALU.add)
                nc.sync.dma_start(ov_d.ap()[i], vfin[:])

            # ---- software-pipelined emission ----------------------------
            pending_scores = []   # (state, g) lagging one group behind u
            pending_tails = []    # states lagging one row behind
            for i in range(BPC):
                T = T_seq[i]
                G = (T + 3) // 4
                ht, logm = emit_row_loads(i, T)
                ps = psp.tile([97, GSZ], F32, tag="ps")
                st = (i, T, ht, logm, ps, {}, {})
                for g in range(G):
                    emit_u(st, g)
                    emit_tanh(st, g)
                    if pending_scores:
                        emit_scores(*pending_scores.pop(0))
                    pending_scores.append((st, g))
                if len(pending_tails) >= 1:
                    emit_tail(pending_tails.pop(0))
                pending_tails.append(st)
            for sg in pending_scores:
                emit_scores(*sg)
            for st in pending_tails:
                emit_tail(st)

    return nc


_NC_CACHE = {}


def _get_nc(T_seq, full):
    key = (T_seq, full)
    if key not in _NC_CACHE:
        _NC_CACHE[key] = build(T_seq, full)
    return _NC_CACHE[key]


def _prep_in_maps(short_perference, current_perference, W, bvec, length_input,
                  perm, T_seq):
    h = np.asarray(short_perference, dtype=np.float32)[0]      # [B, L, H]
    us = np.asarray(current_perference, dtype=np.float32)[0]   # [B, H]
    W = np.asarray(W, dtype=np.float32)
    bvec = np.asarray(bvec, dtype=np.float32)
    lens = np.asarray(length_input).astype(np.int64)

    wt = np.zeros((HP, HP), dtype=np.float32)                  # [c, o]
    wt[:H, :H] = W.T
    wtb0 = wt[0:128].astype(BF16NP)
    wtb1 = wt[128:HP].astype(BF16NP)
    b0 = np.ascontiguousarray(bvec[0:128].reshape(128, 1))
    b1 = np.zeros((128, 1), dtype=np.float32)
    b1[0:H1, 0] = bvec[128:H]

    in_maps = []
    for c in range(N_CORES):
        rows = perm[c]
        hc = np.zeros((BPC, L, HP), dtype=BF16NP)
        hc[:, :, 0:H] = h[rows].astype(BF16NP)
        usc = np.zeros((HP, BPC), dtype=np.float32)
        usc[0:H, :] = us[rows].T
        logm = np.zeros((BPC, NG, GSZ), dtype=np.float32)
        pos = np.arange(NG * GSZ).reshape(NG, GSZ)
        for i, r in enumerate(rows):
            logm[i][pos >= lens[r]] = -1e30
        in_maps.append({
            "h": hc,
            "wtb0": wtb0,
            "wtb1": wtb1,
            "usb0": usc[0:128].astype(BF16NP),
            "usb1": usc[128:HP].astype(BF16NP),
            "b0": b0,
            "b1": b1,
            "logm": logm.astype(BF16NP),
        })
    return in_maps


def run(short_perference, current_perference, W, b, length_input,
        trace=False, **run_kwargs):
    lens = np.asarray(length_input).astype(np.int64)
    perm, T_seq, full = _schedule(lens)
    nc = _get_nc(T_seq, full)
    in_maps = _prep_in_maps(short_perference, current_perference, W, b,
                            lens, perm, T_seq)
    res = run_bass_kernel_spmd(nc, in_maps, list(range(N_CORES)),
                               trace=trace, **run_kwargs)
    v = np.zeros((B, H), dtype=np.float32)
    for c in range(N_CORES):
        ov = np.asarray(res.results[c]["ov"], dtype=np.float32)  # [BPC,128,2]
        oa = np.asarray(res.results[c]["oa"]).astype(np.float32)  # [BPC,4,512]
        for i, r in enumerate(perm[c]):
            T = T_seq[i]
            G = (T + 3) // 4
            denom = 0.0
            for g in range(G):
                N = min(GSZ, 128 * T - g * GSZ)
                denom += oa[i, g, 0:N].sum()
            num = np.concatenate([ov[i, :, 0], ov[i, 0:H1, 1]])
            v[r] = num / denom
    return v, res


def kernel(short_perference, current_perference, W, b, current_batch,
           length_input):
    v, _ = run(short_perference, current_perference, W, b, length_input)
    return v.astype(np.float32)
